# revision 8
# baseline (speedup 1.0000x reference)
"""Multi-head attention (B=4, S=2048, D=1024, H=16) on 8 trn2 NeuronCores.

Sharding: core c -> (batch b = c//2, head-group hg = c%2 of 8 heads).
Each core computes q/k/v projections for its 8 heads, attention, and a
partial output projection (its heads' contribution). Host sums the two
partials per batch and adds b_O.

Per-core device pipeline (all matmuls fp32r or bf16 at 1 cycle/row):
  1. projections: qT/kT [512,2048] fp32r (head-pair stacked on partitions),
     vT [512,2048] bf16
  2. v-hat: PE-transpose vT -> [128, 8, 65] bf16 per k-tile, ones column
     appended (gives softmax denominator Z for free in the PV matmul)
  3. per (head-pair, q-half): scoresT = kT.T @ qT (two heads row-packed,
     K=64), ACT exp(scale=1/8) -> PT bf16, PV accumulate out_unT[65, q]
  4. normalize: recipZ -> PE K=1 broadcast matmul -> multiply -> attn_outT
     [512, 2048] fp32r
  5. output projection: attn_outT.T @ Wo -> partial [2048, 1024] fp32
"""
import sys

if '/opt/trn_rl_repo' not in sys.path:
    sys.path.insert(0, '/opt/trn_rl_repo')

import numpy as np

import concourse.bass as bass  # noqa: F401
import concourse.tile as tile
from concourse import bacc, mybir
from concourse.bass_utils import run_bass_kernel_spmd

N_CORES = 8
B, S, D = 4, 2048, 1024
H = 16
DH = 64                 # head dim
HC = 8                  # heads per core
C = HC * DH             # per-core projection width = 512
F32 = mybir.dt.float32
F32R = mybir.dt.float32r
BF16 = mybir.dt.bfloat16

NKT = S // 128          # 16 k-tiles of 128 along sequence
NM = C // 128           # 4 c-tiles (head pairs)
NDK = D // 128          # 8 contraction tiles for projections
SCALE = 1.0 / np.sqrt(DH)


def round_fp32r(x):
    b = np.ascontiguousarray(x, dtype=np.float32).view(np.uint32)
    b = (b + 0x800) & np.uint32(0xFFFFF000)
    return b.view(np.float32)


def build():
    nc = bacc.Bacc("TRN2", target_bir_lowering=False, debug=False,
                   num_devices=N_CORES)
    XqT = nc.dram_tensor("XqT", [D, S], F32R, kind="ExternalInput").ap()
    XkT = nc.dram_tensor("XkT", [D, S], F32R, kind="ExternalInput").ap()
    XvT = nc.dram_tensor("XvT", [D, S], F32R, kind="ExternalInput").ap()
    Wq = nc.dram_tensor("Wq", [D, C], F32R, kind="ExternalInput").ap()
    Wk = nc.dram_tensor("Wk", [D, C], F32R, kind="ExternalInput").ap()
    Wv = nc.dram_tensor("Wv", [D, C], F32R, kind="ExternalInput").ap()
    Wo = nc.dram_tensor("Wo", [C, D], F32R, kind="ExternalInput").ap()
    bq = nc.dram_tensor("bq", [C], F32, kind="ExternalInput").ap()
    bk = nc.dram_tensor("bk", [C], F32, kind="ExternalInput").ap()
    bv = nc.dram_tensor("bv", [C], F32, kind="ExternalInput").ap()
    OP = nc.dram_tensor("OP", [S, D], F32, kind="ExternalOutput").ap()

    with tile.TileContext(nc) as tc:
        _build_body(nc, tc, XqT, XkT, XvT, Wq, Wk, Wv, Wo, bq, bk, bv, OP)
    nc.compile()
    return nc


def _build_body(nc, tc, XqT, XkT, XvT, Wq, Wk, Wv, Wo, bq, bk, bv, OP):
    from contextlib import ExitStack
    with ExitStack() as stack:
        consts = stack.enter_context(tc.tile_pool(name="consts", bufs=1))
        qkp = stack.enter_context(tc.tile_pool(name="qk", bufs=2 * NM))
        vhp = stack.enter_context(tc.tile_pool(name="vh", bufs=NKT))
        aop = stack.enter_context(tc.tile_pool(name="aout", bufs=NM))

        # constants
        ones_f32 = consts.tile([128, 1], F32)
        nc.vector.memset(ones_f32, 1.0)
        ones_row = consts.tile([1, DH], F32R)
        with nc.allow_low_precision(reason="exact ones to fp32r"):
            nc.vector.tensor_copy(ones_row, ones_f32[0:1, :].broadcast_to((1, DH)))

        bias_t = consts.tile([128, 3 * NM], F32)
        for i, b_ in enumerate((bq, bk, bv)):
            nc.sync.dma_start(
                out=bias_t[:, i * NM:(i + 1) * NM],
                in_=b_.rearrange("(m p) -> p m", p=128))

        # ---------------- phase 1: projections ----------------
        qT = [None] * NM
        kT = [None] * NM
        vT = [None] * NM
        vhat = [None] * NKT
        with ExitStack() as pstack:
            xtp = pstack.enter_context(tc.tile_pool(name="xt", bufs=9))
            wp = pstack.enter_context(tc.tile_pool(name="w", bufs=9))
            pjp = pstack.enter_context(
                tc.tile_pool(name="pj", bufs=3, space="PSUM"))

            for m in range(NM):
                qT[m] = qkp.tile([128, S], F32R, tag="qk", name=f"qTt{m}")
                kT[m] = qkp.tile([128, S], F32R, tag="qk", name=f"kTt{m}")

            # b_v broadcast [128, C] for the natural-layout v epilogue
            bvb = wp.tile([128, C], F32, tag="bvb", bufs=1)
            nc.gpsimd.dma_start(
                out=bvb,
                in_=bass.AP(tensor=bv.tensor, offset=0, ap=[[0, 128], [1, C]]))

            def projection(XT, W, bcol, outs, out_dt):
                for half in range(2):
                    xts = []
                    for kt in range(NDK):
                        xt = xtp.tile([128, S // 2], F32R, tag="xt")
                        nc.sync.dma_start(
                            out=xt,
                            in_=XT[kt * 128:(kt + 1) * 128,
                                   half * (S // 2):(half + 1) * (S // 2)])
                        xts.append(xt)
                    ws = []
                    for kt in range(NDK):
                        w = wp.tile([128, C], F32R, tag="w")
                        nc.sync.dma_start(
                            out=w, in_=W[kt * 128:(kt + 1) * 128, :])
                        ws.append(w)
                    for m in range(NM):
                        for sc in range(2):
                            ps = pjp.tile([128, 512], F32, tag="pj")
                            for kt in range(NDK):
                                nc.tensor.matmul(
                                    ps,
                                    ws[kt][:, m * 128:(m + 1) * 128],
                                    xts[kt][:, sc * 512:(sc + 1) * 512],
                                    start=(kt == 0), stop=(kt == NDK - 1))
                            s0 = half * (S // 2) + sc * 512
                            with nc.allow_low_precision(reason="proj epilogue"):
                                nc.scalar.activation(
                                    out=outs[m][:, s0:s0 + 512], in_=ps,
                                    func=mybir.ActivationFunctionType.Identity,
                                    bias=bias_t[:, bcol + m:bcol + m + 1],
                                    scale=1.0)

            # v in natural [s, c] layout: lhsT = XvT tile, rhs = Wv
            for half in range(2):
                xts = []
                for kt in range(NDK):
                    xt = xtp.tile([128, S // 2], F32R, tag="xt", name=f"xvt{half}_{kt}")
                    nc.sync.dma_start(
                        out=xt,
                        in_=XvT[kt * 128:(kt + 1) * 128,
                                half * (S // 2):(half + 1) * (S // 2)])
                    xts.append(xt)
                ws = []
                for kt in range(NDK):
                    w = wp.tile([128, C], F32R, tag="w", name=f"wv{half}_{kt}")
                    nc.sync.dma_start(out=w, in_=Wv[kt * 128:(kt + 1) * 128, :])
                    ws.append(w)
                for stl in range(8):
                    st = half * 8 + stl
                    ps = pjp.tile([128, C], F32, tag="pj", name=f"vps{st}")
                    for kt in range(NDK):
                        nc.tensor.matmul(
                            ps,
                            xts[kt][:, stl * 128:(stl + 1) * 128],
                            ws[kt],
                            start=(kt == 0), stop=(kt == NDK - 1))
                    vh = vhp.tile([128, HC, DH + 1], F32R, tag="vh",
                                  name=f"vhat{st}")
                    with nc.allow_low_precision(reason="v epilogue"):
                        nc.vector.tensor_add(
                            vh[:, :, 0:DH],
                            ps.rearrange("p (h d) -> p h d", h=HC),
                            bvb.rearrange("p (h d) -> p h d", h=HC))
                        nc.vector.tensor_copy(
                            vh[:, :, DH], ones_f32.broadcast_to((128, HC)))
                    vhat[st] = vh

            projection(XkT, Wk, NM, kT, F32R)
            projection(XqT, Wq, 0, qT, F32R)

        # ---------------- phase 2: attention ----------------
        attn_outT = [None] * NM
        for m in range(NM):
            attn_outT[m] = aop.tile([128, S], F32R, tag="aout", name=f"aoutT{m}")

        with ExitStack() as astack:
            ptp = astack.enter_context(tc.tile_pool(name="pt", bufs=6))
            nrm = astack.enter_context(tc.tile_pool(name="nrm", bufs=8))
            sp = astack.enter_context(
                tc.tile_pool(name="sps", bufs=2, space="PSUM"))
            pvp = astack.enter_context(
                tc.tile_pool(name="pv", bufs=4, space="PSUM"))

            for hp in range(NM):
                for qh in range(2):
                    q0 = qh * (S // 2)
                    pv = [pvp.tile([DH + 1, 512], F32, tag="pv",
                                    name=f"pv{hp}_{qh}_{i}")
                          for i in range(4)]  # [headA q0, A q1, B q0, B q1]
                    for kt in range(NKT):
                        pts = []
                        for hh in range(2):
                            h = 2 * hp + hh
                            dlo = hh * DH
                            sps = sp.tile([128, 1024], F32, tag="sps")
                            for qc in range(2):
                                nc.tensor.matmul(
                                    sps[:, qc * 512:(qc + 1) * 512],
                                    kT[hp][dlo:dlo + DH,
                                           kt * 128:(kt + 1) * 128],
                                    qT[hp][dlo:dlo + DH,
                                           q0 + qc * 512:q0 + (qc + 1) * 512],
                                    start=True, stop=True)
                            pt = ptp.tile([128, 1024], F32R, tag="pt")
                            nc.scalar.activation(
                                out=pt, in_=sps,
                                func=mybir.ActivationFunctionType.Exp,
                                scale=float(SCALE))
                            pts.append(pt)
                        for hh in range(2):
                            h = 2 * hp + hh
                            for qc in range(2):
                                nc.tensor.matmul(
                                    pv[2 * hh + qc],
                                    vhat[kt][:, h, :],
                                    pts[hh][:, qc * 512:(qc + 1) * 512],
                                    start=(kt == 0), stop=(kt == NKT - 1))
                    # normalize + write attn_outT
                    for hh in range(2):
                        dlo = hh * DH
                        for qc in range(2):
                            acc = pv[2 * hh + qc]
                            rz = nrm.tile([1, 512], F32R, tag="rz")
                            with nc.allow_low_precision(reason="recipZ"):
                                nc.vector.reciprocal(
                                    out=rz, in_=acc[DH:DH + 1, :])
                            bc_ps = sp.tile([DH, 512], F32, tag="sps")
                            nc.tensor.matmul(bc_ps, ones_row, rz,
                                             start=True, stop=True)
                            bc = nrm.tile([DH, 512], F32, tag="bc")
                            nc.vector.tensor_copy(bc, bc_ps)
                            s0 = q0 + qc * 512
                            with nc.allow_low_precision(reason="attn_outT"):
                                nc.vector.tensor_mul(
                                    attn_outT[hp][dlo:dlo + DH, s0:s0 + 512],
                                    acc[0:DH, :], bc)

        # ---------------- phase 3: output projection ----------------
        with ExitStack() as ostack:
            ost = ostack.enter_context(tc.tile_pool(name="ost", bufs=4))
            wop = ostack.enter_context(tc.tile_pool(name="wo", bufs=NM))
            osp = ostack.enter_context(
                tc.tile_pool(name="osp", bufs=2, space="PSUM"))
            wo_tiles = []
            for m in range(NM):
                w = wop.tile([128, D], F32R, tag="wo")
                nc.sync.dma_start(out=w, in_=Wo[m * 128:(m + 1) * 128, :])
                wo_tiles.append(w)
            for st in range(NKT):
                for oc in range(2):
                    ps = osp.tile([128, 512], F32, tag="osp")
                    for ct in range(NM):
                        nc.tensor.matmul(
                            ps,
                            attn_outT[ct][:, st * 128:(st + 1) * 128],
                            wo_tiles[ct][:, oc * 512:(oc + 1) * 512],
                            start=(ct == 0), stop=(ct == NM - 1))
                    ot = ost.tile([128, 512], F32, tag="ost")
                    nc.scalar.activation(
                        out=ot, in_=ps,
                        func=mybir.ActivationFunctionType.Identity,
                        bias=0.0, scale=1.0)
                    nc.sync.dma_start(
                        out=OP[st * 128:(st + 1) * 128,
                               oc * 512:(oc + 1) * 512],
                        in_=ot)


_NC_CACHE = None


def _get_nc():
    global _NC_CACHE
    if _NC_CACHE is None:
        _NC_CACHE = build()
    return _NC_CACHE


def kernel(Q, K, V, W_Q, b_Q, W_K, b_K, W_V, b_V, W_O, b_O):
    Q = np.asarray(Q, dtype=np.float32)
    K = np.asarray(K, dtype=np.float32)
    V = np.asarray(V, dtype=np.float32)
    nc = _get_nc()

    XqTs = [round_fp32r(Q[b].T) for b in range(B)]
    XkTs = [round_fp32r(K[b].T) for b in range(B)]
    XvTs = [round_fp32r(V[b].T) for b in range(B)]
    Wqs = [round_fp32r(np.asarray(W_Q)[:, hg * C:(hg + 1) * C]) for hg in range(2)]
    Wks = [round_fp32r(np.asarray(W_K)[:, hg * C:(hg + 1) * C]) for hg in range(2)]
    Wvs = [round_fp32r(np.asarray(W_V)[:, hg * C:(hg + 1) * C]) for hg in range(2)]
    Wos = [round_fp32r(np.asarray(W_O)[hg * C:(hg + 1) * C, :]) for hg in range(2)]
    bqs = [np.ascontiguousarray(np.asarray(b_Q, dtype=np.float32)[hg * C:(hg + 1) * C]) for hg in range(2)]
    bks = [np.ascontiguousarray(np.asarray(b_K, dtype=np.float32)[hg * C:(hg + 1) * C]) for hg in range(2)]
    bvs = [np.ascontiguousarray(np.asarray(b_V, dtype=np.float32)[hg * C:(hg + 1) * C]) for hg in range(2)]

    in_maps = []
    for c in range(N_CORES):
        b, hg = c // 2, c % 2
        in_maps.append({
            "XqT": XqTs[b], "XkT": XkTs[b], "XvT": XvTs[b],
            "Wq": Wqs[hg], "Wk": Wks[hg], "Wv": Wvs[hg], "Wo": Wos[hg],
            "bq": bqs[hg], "bk": bks[hg], "bv": bvs[hg],
        })
    global _last_in_maps
    _last_in_maps = in_maps
    res = run_bass_kernel_spmd(nc, in_maps, list(range(N_CORES)))
    out = np.empty((B, S, D), dtype=np.float32)
    bO = np.asarray(b_O, dtype=np.float32)
    for b in range(B):
        out[b] = res.results[2 * b]["OP"] + res.results[2 * b + 1]["OP"] + bO
    return out


# revision 9
# speedup vs baseline: 1.1057x; 1.1057x over previous
"""Multi-head attention (B=4, S=2048, D=1024, H=16) on 8 trn2 NeuronCores.

Sharding: core c -> (batch b = c//2, head-group hg = c%2 of 8 heads).
Each core computes q/k/v projections for its 8 heads, attention, and a
partial output projection (its heads' contribution). Host sums the two
partials per batch and adds b_O.

Per-core device pipeline:
  1. projections: qT/kT [512,2048] (head-pair stacked on partitions),
     v-hat [128, 8, 65] in natural [s,c] layout (lhsT=XvT tile, rhs=Wv)
     with a ones column appended (softmax Z falls out of the PV matmul)
  2. per (head-pair, q-half): scoresT = kT.T @ qT (two heads row-packed,
     K=64), ACT exp(scale=1/8) -> PT, PV accumulates out_unT[65, q]
  3. tail: stage psum->SBUF (frees PSUM fast), recipZ, PE K=1 broadcast
     matmul, multiply -> attn_outT [512, 2048]
  4. output projection: attn_outT.T @ Wo -> partial [2048, 1024] fp32
"""
import sys

if '/opt/trn_rl_repo' not in sys.path:
    sys.path.insert(0, '/opt/trn_rl_repo')

import ml_dtypes
import numpy as np

import concourse.bass as bass
import concourse.tile as tile
from concourse import bacc, mybir
from concourse.bass_utils import run_bass_kernel_spmd

N_CORES = 8
B, S, D = 4, 2048, 1024
H = 16
DH = 64                 # head dim
HC = 8                  # heads per core
C = HC * DH             # per-core projection width = 512
F32 = mybir.dt.float32
F32R = mybir.dt.float32r
BF16 = mybir.dt.bfloat16

NKT = S // 128          # 16 s-tiles of 128
NM = C // 128           # 4 c-tiles (head pairs)
NDK = D // 128          # 8 contraction tiles for projections
SCALE = 1.0 / np.sqrt(DH)

# dtype config for the four matmul stages (BF16 or F32R)
PROJ_DT = BF16          # q/k/v projection inputs (XT, W)
QK_DT = BF16            # qT/kT tiles (scores matmul inputs)
PV_DT = BF16            # PT + v-hat (PV matmul inputs)
OUT_DT = BF16           # attn_outT + Wo (output projection inputs)


def round_fp32r(x):
    b = np.ascontiguousarray(x, dtype=np.float32).view(np.uint32)
    b = (b + 0x800) & np.uint32(0xFFFFF000)
    return b.view(np.float32)


def prep(x, dt):
    if dt == BF16:
        return np.ascontiguousarray(x).astype(ml_dtypes.bfloat16)
    return round_fp32r(x)


def build():
    nc = bacc.Bacc("TRN2", target_bir_lowering=False, debug=False,
                   num_devices=N_CORES)
    XqT = nc.dram_tensor("XqT", [D, S], PROJ_DT, kind="ExternalInput").ap()
    XkT = nc.dram_tensor("XkT", [D, S], PROJ_DT, kind="ExternalInput").ap()
    XvT = nc.dram_tensor("XvT", [D, S], PROJ_DT, kind="ExternalInput").ap()
    Wq = nc.dram_tensor("Wq", [D, C], PROJ_DT, kind="ExternalInput").ap()
    Wk = nc.dram_tensor("Wk", [D, C], PROJ_DT, kind="ExternalInput").ap()
    Wv = nc.dram_tensor("Wv", [D, C], PROJ_DT, kind="ExternalInput").ap()
    Wo = nc.dram_tensor("Wo", [C, D], OUT_DT, kind="ExternalInput").ap()
    bq = nc.dram_tensor("bq", [C], F32, kind="ExternalInput").ap()
    bk = nc.dram_tensor("bk", [C], F32, kind="ExternalInput").ap()
    bv = nc.dram_tensor("bv", [C], F32, kind="ExternalInput").ap()
    OP = nc.dram_tensor("OP", [S, D], F32, kind="ExternalOutput").ap()

    with tile.TileContext(nc) as tc:
        _build_body(nc, tc, XqT, XkT, XvT, Wq, Wk, Wv, Wo, bq, bk, bv, OP)
    nc.compile()
    return nc


def _build_body(nc, tc, XqT, XkT, XvT, Wq, Wk, Wv, Wo, bq, bk, bv, OP):
    from contextlib import ExitStack
    with ExitStack() as stack:
        consts = stack.enter_context(tc.tile_pool(name="consts", bufs=1))
        qkp = stack.enter_context(tc.tile_pool(name="qk", bufs=2 * NM))
        vhp = stack.enter_context(tc.tile_pool(name="vh", bufs=NKT))
        aop = stack.enter_context(tc.tile_pool(name="aout", bufs=NM))

        # constants
        ones_f32 = consts.tile([128, 1], F32)
        nc.vector.memset(ones_f32, 1.0)
        ones_row = consts.tile([1, DH], F32R)
        with nc.allow_low_precision(reason="exact ones to fp32r"):
            nc.vector.tensor_copy(
                ones_row, ones_f32[0:1, :].broadcast_to((1, DH)))

        bias_t = consts.tile([128, 2 * NM], F32)
        for i, b_ in enumerate((bq, bk)):
            nc.sync.dma_start(
                out=bias_t[:, i * NM:(i + 1) * NM],
                in_=b_.rearrange("(m p) -> p m", p=128))
        bvb = consts.tile([128, C], F32)
        nc.gpsimd.dma_start(
            out=bvb,
            in_=bass.AP(tensor=bv.tensor, offset=0, ap=[[0, 128], [1, C]]))

        # ---------------- phase 1: projections ----------------
        qT = [None] * NM
        kT = [None] * NM
        vhat = [None] * NKT
        with ExitStack() as pstack:
            xtp = pstack.enter_context(tc.tile_pool(name="xt", bufs=12))
            wp = pstack.enter_context(tc.tile_pool(name="w", bufs=10))
            pjp = pstack.enter_context(
                tc.tile_pool(name="pj", bufs=3, space="PSUM"))

            for m in range(NM):
                qT[m] = qkp.tile([128, S], QK_DT, tag="qk", name=f"qTt{m}")
                kT[m] = qkp.tile([128, S], QK_DT, tag="qk", name=f"kTt{m}")

            # v in natural [s, c] layout: lhsT = XvT tile, rhs = Wv
            for half in range(2):
                xts = []
                for kt in range(NDK):
                    xt = xtp.tile([128, S // 2], PROJ_DT, tag="xt",
                                  name=f"xvt{half}_{kt}")
                    nc.sync.dma_start(
                        out=xt,
                        in_=XvT[kt * 128:(kt + 1) * 128,
                                half * (S // 2):(half + 1) * (S // 2)])
                    xts.append(xt)
                ws = []
                for kt in range(NDK):
                    w = wp.tile([128, C], PROJ_DT, tag="w",
                                name=f"wv{half}_{kt}")
                    nc.sync.dma_start(out=w, in_=Wv[kt * 128:(kt + 1) * 128, :])
                    ws.append(w)
                for stl in range(8):
                    st = half * 8 + stl
                    ps = pjp.tile([128, C], F32, tag="pj", name=f"vps{st}")
                    for kt in range(NDK):
                        nc.tensor.matmul(
                            ps,
                            xts[kt][:, stl * 128:(stl + 1) * 128],
                            ws[kt],
                            start=(kt == 0), stop=(kt == NDK - 1))
                    vh = vhp.tile([128, HC, DH + 1], PV_DT, tag="vh",
                                  name=f"vhat{st}")
                    with nc.allow_low_precision(reason="v epilogue"):
                        nc.vector.tensor_add(
                            vh[:, :, 0:DH],
                            ps.rearrange("p (h d) -> p h d", h=HC),
                            bvb.rearrange("p (h d) -> p h d", h=HC))
                        nc.vector.tensor_copy(
                            vh[:, :, DH], ones_f32.broadcast_to((128, HC)))
                    vhat[st] = vh

            def projection(XT, W, bcol, outs):
                for half in range(2):
                    xts = []
                    for kt in range(NDK):
                        xt = xtp.tile([128, S // 2], PROJ_DT, tag="xt")
                        nc.sync.dma_start(
                            out=xt,
                            in_=XT[kt * 128:(kt + 1) * 128,
                                   half * (S // 2):(half + 1) * (S // 2)])
                        xts.append(xt)
                    ws = []
                    for kt in range(NDK):
                        w = wp.tile([128, C], PROJ_DT, tag="w")
                        nc.sync.dma_start(
                            out=w, in_=W[kt * 128:(kt + 1) * 128, :])
                        ws.append(w)
                    for m in range(NM):
                        for sc in range(2):
                            ps = pjp.tile([128, 512], F32, tag="pj")
                            for kt in range(NDK):
                                nc.tensor.matmul(
                                    ps,
                                    ws[kt][:, m * 128:(m + 1) * 128],
                                    xts[kt][:, sc * 512:(sc + 1) * 512],
                                    start=(kt == 0), stop=(kt == NDK - 1))
                            s0 = half * (S // 2) + sc * 512
                            with nc.allow_low_precision(reason="proj epi"):
                                nc.scalar.activation(
                                    out=outs[m][:, s0:s0 + 512], in_=ps,
                                    func=mybir.ActivationFunctionType.Identity,
                                    bias=bias_t[:, bcol + m:bcol + m + 1],
                                    scale=1.0)

            projection(XkT, Wk, NM, kT)
            projection(XqT, Wq, 0, qT)

        # ---------------- phase 2: attention ----------------
        attn_outT = [None] * NM
        for m in range(NM):
            attn_outT[m] = aop.tile([128, S], OUT_DT, tag="aout",
                                    name=f"aoutT{m}")

        with ExitStack() as astack:
            ptp = astack.enter_context(tc.tile_pool(name="pt", bufs=6))
            stg = astack.enter_context(tc.tile_pool(name="stg", bufs=8))
            nrm = astack.enter_context(tc.tile_pool(name="nrm", bufs=8))
            sp = astack.enter_context(
                tc.tile_pool(name="sps", bufs=2, space="PSUM"))
            pvp = astack.enter_context(
                tc.tile_pool(name="pv", bufs=4, space="PSUM"))

            for hp in range(NM):
                for qh in range(2):
                    q0 = qh * (S // 2)
                    pv = [pvp.tile([DH + 1, 512], F32, tag="pv",
                                   name=f"pv{hp}_{qh}_{i}")
                          for i in range(4)]  # [A q0, A q1, B q0, B q1]
                    for kt in range(NKT):
                        pts = []
                        for hh in range(2):
                            dlo = hh * DH
                            sps = sp.tile([128, 1024], F32, tag="sps")
                            for qc in range(2):
                                nc.tensor.matmul(
                                    sps[:, qc * 512:(qc + 1) * 512],
                                    kT[hp][dlo:dlo + DH,
                                           kt * 128:(kt + 1) * 128],
                                    qT[hp][dlo:dlo + DH,
                                           q0 + qc * 512:q0 + (qc + 1) * 512],
                                    start=True, stop=True)
                            pt = ptp.tile([128, 1024], PV_DT, tag="pt")
                            nc.scalar.activation(
                                out=pt, in_=sps,
                                func=mybir.ActivationFunctionType.Exp,
                                scale=float(SCALE))
                            pts.append(pt)
                        for hh in range(2):
                            h = 2 * hp + hh
                            for qc in range(2):
                                nc.tensor.matmul(
                                    pv[2 * hh + qc],
                                    vhat[kt][:, h, :],
                                    pts[hh][:, qc * 512:(qc + 1) * 512],
                                    start=(kt == 0), stop=(kt == NKT - 1))
                    # tails: stage out of PSUM fast, then normalize
                    for hh in range(2):
                        dlo = hh * DH
                        for qc in range(2):
                            acc = pv[2 * hh + qc]
                            st_t = stg.tile([DH + 1, 512], F32R, tag="stg")
                            with nc.allow_low_precision(reason="stage"):
                                nc.vector.tensor_copy(st_t, acc)
                            rz = nrm.tile([1, 512], F32R, tag="rz")
                            with nc.allow_low_precision(reason="recipZ"):
                                nc.vector.reciprocal(
                                    out=rz, in_=st_t[DH:DH + 1, :])
                            bc_ps = sp.tile([DH, 512], F32, tag="sps")
                            nc.tensor.matmul(bc_ps, ones_row, rz,
                                             start=True, stop=True)
                            bc = nrm.tile([DH, 512], F32R, tag="bc")
                            with nc.allow_low_precision(reason="bc"):
                                nc.vector.tensor_copy(bc, bc_ps)
                            s0 = q0 + qc * 512
                            with nc.allow_low_precision(reason="attn_outT"):
                                nc.vector.tensor_mul(
                                    attn_outT[hp][dlo:dlo + DH, s0:s0 + 512],
                                    st_t[0:DH, :], bc)

        # ---------------- phase 3: output projection ----------------
        with ExitStack() as ostack:
            ost = ostack.enter_context(tc.tile_pool(name="ost", bufs=4))
            wop = ostack.enter_context(tc.tile_pool(name="wo", bufs=NM))
            osp = ostack.enter_context(
                tc.tile_pool(name="osp", bufs=2, space="PSUM"))
            wo_tiles = []
            for m in range(NM):
                w = wop.tile([128, D], OUT_DT, tag="wo", name=f"wo{m}")
                nc.sync.dma_start(out=w, in_=Wo[m * 128:(m + 1) * 128, :])
                wo_tiles.append(w)
            for st in range(NKT):
                for oc in range(2):
                    ps = osp.tile([128, 512], F32, tag="osp")
                    for ct in range(NM):
                        nc.tensor.matmul(
                            ps,
                            attn_outT[ct][:, st * 128:(st + 1) * 128],
                            wo_tiles[ct][:, oc * 512:(oc + 1) * 512],
                            start=(ct == 0), stop=(ct == NM - 1))
                    ot = ost.tile([128, 512], F32, tag="ost")
                    nc.scalar.activation(
                        out=ot, in_=ps,
                        func=mybir.ActivationFunctionType.Identity,
                        bias=0.0, scale=1.0)
                    nc.sync.dma_start(
                        out=OP[st * 128:(st + 1) * 128,
                               oc * 512:(oc + 1) * 512],
                        in_=ot)


_NC_CACHE = None
_last_in_maps = None


def _get_nc():
    global _NC_CACHE
    if _NC_CACHE is None:
        _NC_CACHE = build()
    return _NC_CACHE


def kernel(Q, K, V, W_Q, b_Q, W_K, b_K, W_V, b_V, W_O, b_O):
    global _last_in_maps
    Q = np.asarray(Q, dtype=np.float32)
    K = np.asarray(K, dtype=np.float32)
    V = np.asarray(V, dtype=np.float32)
    nc = _get_nc()

    XqTs = [prep(Q[b].T, PROJ_DT) for b in range(B)]
    XkTs = [prep(K[b].T, PROJ_DT) for b in range(B)]
    XvTs = [prep(V[b].T, PROJ_DT) for b in range(B)]
    Wqs = [prep(np.asarray(W_Q)[:, hg * C:(hg + 1) * C], PROJ_DT)
           for hg in range(2)]
    Wks = [prep(np.asarray(W_K)[:, hg * C:(hg + 1) * C], PROJ_DT)
           for hg in range(2)]
    Wvs = [prep(np.asarray(W_V)[:, hg * C:(hg + 1) * C], PROJ_DT)
           for hg in range(2)]
    Wos = [prep(np.asarray(W_O)[hg * C:(hg + 1) * C, :], OUT_DT)
           for hg in range(2)]
    bqs = [np.ascontiguousarray(np.asarray(b_Q, dtype=np.float32)[hg * C:(hg + 1) * C])
           for hg in range(2)]
    bks = [np.ascontiguousarray(np.asarray(b_K, dtype=np.float32)[hg * C:(hg + 1) * C])
           for hg in range(2)]
    bvs = [np.ascontiguousarray(np.asarray(b_V, dtype=np.float32)[hg * C:(hg + 1) * C])
           for hg in range(2)]

    in_maps = []
    for c in range(N_CORES):
        b, hg = c // 2, c % 2
        in_maps.append({
            "XqT": XqTs[b], "XkT": XkTs[b], "XvT": XvTs[b],
            "Wq": Wqs[hg], "Wk": Wks[hg], "Wv": Wvs[hg], "Wo": Wos[hg],
            "bq": bqs[hg], "bk": bks[hg], "bv": bvs[hg],
        })
    _last_in_maps = in_maps
    res = run_bass_kernel_spmd(nc, in_maps, list(range(N_CORES)))
    out = np.empty((B, S, D), dtype=np.float32)
    bO = np.asarray(b_O, dtype=np.float32)
    for b in range(B):
        out[b] = res.results[2 * b]["OP"] + res.results[2 * b + 1]["OP"] + bO
    return out


# revision 10
# speedup vs baseline: 1.3038x; 1.1792x over previous
"""Multi-head attention (B=4, S=2048, D=1024, H=16) on 8 trn2 NeuronCores.

Sharding: core c -> (batch b = c//2, head-group hg = c%2 of 8 heads).
Each core computes q/k/v projections for its 8 heads, attention, and a
partial output projection (its heads' contribution). Host sums the two
partials per batch and adds b_O.

Per-core device pipeline:
  1. projections: qT/kT [512,2048] (head-pair stacked on partitions),
     v-hat [128, 8, 65] in natural [s,c] layout (lhsT=XvT tile, rhs=Wv)
     with a ones column appended (softmax Z falls out of the PV matmul)
  2. per (head-pair, q-half): scoresT = kT.T @ qT (two heads row-packed,
     K=64), ACT exp(scale=1/8) -> PT, PV accumulates out_unT[65, q]
  3. tail: stage psum->SBUF (frees PSUM fast), recipZ, PE K=1 broadcast
     matmul, multiply -> attn_outT [512, 2048]
  4. output projection: attn_outT.T @ Wo -> partial [2048, 1024] fp32
"""
import sys

if '/opt/trn_rl_repo' not in sys.path:
    sys.path.insert(0, '/opt/trn_rl_repo')

import ml_dtypes
import numpy as np

import concourse.bass as bass
import concourse.tile as tile
from concourse import bacc, mybir
from concourse.bass_utils import run_bass_kernel_spmd

N_CORES = 8
B, S, D = 4, 2048, 1024
H = 16
DH = 64                 # head dim
HC = 8                  # heads per core
C = HC * DH             # per-core projection width = 512
F32 = mybir.dt.float32
F32R = mybir.dt.float32r
BF16 = mybir.dt.bfloat16

NKT = S // 128          # 16 s-tiles of 128
NM = C // 128           # 4 c-tiles (head pairs)
NDK = D // 128          # 8 contraction tiles for projections
SCALE = 1.0 / np.sqrt(DH)

# dtype config for the four matmul stages (BF16 or F32R)
PROJ_DT = BF16          # q/k/v projection inputs (XT, W)
QK_DT = BF16            # qT/kT tiles (scores matmul inputs)
PV_DT = BF16            # PT + v-hat (PV matmul inputs)
OUT_DT = BF16           # attn_outT + Wo (output projection inputs)


def round_fp32r(x):
    b = np.ascontiguousarray(x, dtype=np.float32).view(np.uint32)
    b = (b + 0x800) & np.uint32(0xFFFFF000)
    return b.view(np.float32)


def prep(x, dt):
    if dt == BF16:
        return np.ascontiguousarray(x).astype(ml_dtypes.bfloat16)
    return round_fp32r(x)


def build():
    nc = bacc.Bacc("TRN2", target_bir_lowering=False, debug=False,
                   num_devices=N_CORES)
    XqT = nc.dram_tensor("XqT", [D, S], PROJ_DT, kind="ExternalInput").ap()
    XkT = nc.dram_tensor("XkT", [D, S], PROJ_DT, kind="ExternalInput").ap()
    XvT = nc.dram_tensor("XvT", [D, S], PROJ_DT, kind="ExternalInput").ap()
    Wq = nc.dram_tensor("Wq", [D, C], PROJ_DT, kind="ExternalInput").ap()
    Wk = nc.dram_tensor("Wk", [D, C], PROJ_DT, kind="ExternalInput").ap()
    Wv = nc.dram_tensor("Wv", [D, C], PROJ_DT, kind="ExternalInput").ap()
    Wo = nc.dram_tensor("Wo", [C, D], OUT_DT, kind="ExternalInput").ap()
    bq = nc.dram_tensor("bq", [C], F32, kind="ExternalInput").ap()
    bk = nc.dram_tensor("bk", [C], F32, kind="ExternalInput").ap()
    bv = nc.dram_tensor("bv", [C], F32, kind="ExternalInput").ap()
    OP = nc.dram_tensor("OP", [S, D], F32, kind="ExternalOutput").ap()

    with tile.TileContext(nc) as tc:
        _build_body(nc, tc, XqT, XkT, XvT, Wq, Wk, Wv, Wo, bq, bk, bv, OP)
    nc.compile()
    return nc


def _build_body(nc, tc, XqT, XkT, XvT, Wq, Wk, Wv, Wo, bq, bk, bv, OP):
    from contextlib import ExitStack
    with ExitStack() as stack:
        consts = stack.enter_context(tc.tile_pool(name="consts", bufs=1))
        qkp = stack.enter_context(tc.tile_pool(name="qk", bufs=2 * NM))
        vhp = stack.enter_context(tc.tile_pool(name="vh", bufs=NKT))
        aop = stack.enter_context(tc.tile_pool(name="aout", bufs=NM))

        # constants
        ones_f32 = consts.tile([128, 1], F32)
        nc.vector.memset(ones_f32, 1.0)
        ones_row = consts.tile([1, DH], F32R)
        with nc.allow_low_precision(reason="exact ones to fp32r"):
            nc.vector.tensor_copy(
                ones_row, ones_f32[0:1, :].broadcast_to((1, DH)))

        bias_t = consts.tile([128, 2 * NM], F32)
        for i, b_ in enumerate((bq, bk)):
            nc.sync.dma_start(
                out=bias_t[:, i * NM:(i + 1) * NM],
                in_=b_.rearrange("(m p) -> p m", p=128))
        bvb = consts.tile([128, C], F32)
        nc.gpsimd.dma_start(
            out=bvb,
            in_=bass.AP(tensor=bv.tensor, offset=0, ap=[[0, 128], [1, C]]))

        # ---------------- phase 1: projections ----------------
        qT = [None] * NM
        kT = [None] * NM
        vhat = [None] * NKT
        with ExitStack() as pstack:
            xtp = pstack.enter_context(tc.tile_pool(name="xt", bufs=12))
            wp = pstack.enter_context(tc.tile_pool(name="w", bufs=10))
            pjp = pstack.enter_context(
                tc.tile_pool(name="pj", bufs=3, space="PSUM"))

            for m in range(NM):
                qT[m] = qkp.tile([128, S], QK_DT, tag="qk", name=f"qTt{m}")
                kT[m] = qkp.tile([128, S], QK_DT, tag="qk", name=f"kTt{m}")

            # v in natural [s, c] layout: lhsT = XvT tile, rhs = Wv
            for half in range(2):
                xts = []
                for kt in range(NDK):
                    xt = xtp.tile([128, S // 2], PROJ_DT, tag="xt",
                                  name=f"xvt{half}_{kt}")
                    nc.sync.dma_start(
                        out=xt,
                        in_=XvT[kt * 128:(kt + 1) * 128,
                                half * (S // 2):(half + 1) * (S // 2)])
                    xts.append(xt)
                ws = []
                for kt in range(NDK):
                    w = wp.tile([128, C], PROJ_DT, tag="w",
                                name=f"wv{half}_{kt}")
                    nc.sync.dma_start(out=w, in_=Wv[kt * 128:(kt + 1) * 128, :])
                    ws.append(w)
                for stl in range(8):
                    st = half * 8 + stl
                    ps = pjp.tile([128, C], F32, tag="pj", name=f"vps{st}")
                    for kt in range(NDK):
                        nc.tensor.matmul(
                            ps,
                            xts[kt][:, stl * 128:(stl + 1) * 128],
                            ws[kt],
                            start=(kt == 0), stop=(kt == NDK - 1))
                    vh = vhp.tile([128, HC, DH + 1], PV_DT, tag="vh",
                                  name=f"vhat{st}")
                    with nc.allow_low_precision(reason="v epilogue"):
                        nc.vector.tensor_add(
                            vh[:, :, 0:DH],
                            ps.rearrange("p (h d) -> p h d", h=HC),
                            bvb.rearrange("p (h d) -> p h d", h=HC))
                        nc.vector.tensor_copy(
                            vh[:, :, DH], ones_f32.broadcast_to((128, HC)))
                    vhat[st] = vh

            def projection(XT, W, bcol, outs):
                for half in range(2):
                    xts = []
                    for kt in range(NDK):
                        xt = xtp.tile([128, S // 2], PROJ_DT, tag="xt")
                        nc.sync.dma_start(
                            out=xt,
                            in_=XT[kt * 128:(kt + 1) * 128,
                                   half * (S // 2):(half + 1) * (S // 2)])
                        xts.append(xt)
                    ws = []
                    for kt in range(NDK):
                        w = wp.tile([128, C], PROJ_DT, tag="w")
                        nc.sync.dma_start(
                            out=w, in_=W[kt * 128:(kt + 1) * 128, :])
                        ws.append(w)
                    for m in range(NM):
                        for sc in range(2):
                            ps = pjp.tile([128, 512], F32, tag="pj")
                            for kt in range(NDK):
                                nc.tensor.matmul(
                                    ps,
                                    ws[kt][:, m * 128:(m + 1) * 128],
                                    xts[kt][:, sc * 512:(sc + 1) * 512],
                                    start=(kt == 0), stop=(kt == NDK - 1))
                            s0 = half * (S // 2) + sc * 512
                            with nc.allow_low_precision(reason="proj epi"):
                                nc.scalar.activation(
                                    out=outs[m][:, s0:s0 + 512], in_=ps,
                                    func=mybir.ActivationFunctionType.Identity,
                                    bias=bias_t[:, bcol + m:bcol + m + 1],
                                    scale=1.0)

            projection(XkT, Wk, NM, kT)
            projection(XqT, Wq, 0, qT)

        # ---------------- phase 2: attention ----------------
        attn_outT = [None] * NM
        for m in range(NM):
            attn_outT[m] = aop.tile([128, S], OUT_DT, tag="aout",
                                    name=f"aoutT{m}")

        with ExitStack() as astack:
            ptp = astack.enter_context(tc.tile_pool(name="pt", bufs=6))
            stg = astack.enter_context(tc.tile_pool(name="stg", bufs=6))
            nrm = astack.enter_context(tc.tile_pool(name="nrm", bufs=4))
            wop = astack.enter_context(tc.tile_pool(name="wo", bufs=NM))
            oap = astack.enter_context(tc.tile_pool(name="oacc", bufs=32))
            sp = astack.enter_context(
                tc.tile_pool(name="sps", bufs=2, space="PSUM"))
            pvp = astack.enter_context(
                tc.tile_pool(name="pv", bufs=4, space="PSUM"))

            wo_tiles = []
            for m in range(NM):
                w = wop.tile([128, D], OUT_DT, tag="wo", name=f"wo{m}")
                nc.sync.dma_start(out=w, in_=Wo[m * 128:(m + 1) * 128, :])
                wo_tiles.append(w)
            out_acc = [[None] * 2 for _ in range(NKT)]

            for hp in range(NM):
                for qh in range(2):
                    q0 = qh * (S // 2)
                    pv = [pvp.tile([DH + 1, 512], F32, tag="pv",
                                   name=f"pv{hp}_{qh}_{i}")
                          for i in range(4)]  # [A q0, A q1, B q0, B q1]
                    for kt in range(NKT):
                        pts = []
                        for hh in range(2):
                            dlo = hh * DH
                            sps = sp.tile([128, 1024], F32, tag="sps")
                            for qc in range(2):
                                nc.tensor.matmul(
                                    sps[:, qc * 512:(qc + 1) * 512],
                                    kT[hp][dlo:dlo + DH,
                                           kt * 128:(kt + 1) * 128],
                                    qT[hp][dlo:dlo + DH,
                                           q0 + qc * 512:q0 + (qc + 1) * 512],
                                    start=True, stop=True)
                            pt = ptp.tile([128, 1024], PV_DT, tag="pt")
                            nc.scalar.activation(
                                out=pt, in_=sps,
                                func=mybir.ActivationFunctionType.Exp,
                                scale=float(SCALE))
                            pts.append(pt)
                        for hh in range(2):
                            h = 2 * hp + hh
                            for qc in range(2):
                                nc.tensor.matmul(
                                    pv[2 * hh + qc],
                                    vhat[kt][:, h, :],
                                    pts[hh][:, qc * 512:(qc + 1) * 512],
                                    start=(kt == 0), stop=(kt == NKT - 1))
                    # tails: stage out of PSUM fast, then normalize
                    for hh in range(2):
                        dlo = hh * DH
                        for qc in range(2):
                            acc = pv[2 * hh + qc]
                            st_t = stg.tile([DH + 1, 512], F32R, tag="stg")
                            with nc.allow_low_precision(reason="stage"):
                                nc.vector.tensor_copy(st_t, acc)
                            rz = nrm.tile([1, 512], F32R, tag="rz")
                            with nc.allow_low_precision(reason="recipZ"):
                                nc.vector.reciprocal(
                                    out=rz, in_=st_t[DH:DH + 1, :])
                            bc_ps = sp.tile([DH, 512], F32, tag="sps")
                            nc.tensor.matmul(bc_ps, ones_row, rz,
                                             start=True, stop=True)
                            bc = nrm.tile([DH, 512], F32R, tag="bc")
                            with nc.allow_low_precision(reason="bc"):
                                nc.vector.tensor_copy(bc, bc_ps)
                            s0 = q0 + qc * 512
                            with nc.allow_low_precision(reason="attn_outT"):
                                nc.vector.tensor_mul(
                                    attn_outT[hp][dlo:dlo + DH, s0:s0 + 512],
                                    st_t[0:DH, :], bc)
                    # incremental output projection for this block's s-tiles
                    for stl in range(8):
                        st = qh * 8 + stl
                        for oc in range(2):
                            ps = pvp.tile([128, 512], F32, tag="pv",
                                          name=f"ops{hp}_{st}_{oc}")
                            nc.tensor.matmul(
                                ps,
                                attn_outT[hp][:, st * 128:(st + 1) * 128],
                                wo_tiles[hp][:, oc * 512:(oc + 1) * 512],
                                start=True, stop=True)
                            if hp == 0:
                                oa = oap.tile([128, 512], F32, tag="oacc",
                                              name=f"oacc{st}_{oc}")
                                out_acc[st][oc] = oa
                                nc.vector.tensor_copy(oa, ps)
                            else:
                                oa = out_acc[st][oc]
                                nc.vector.tensor_add(oa, oa, ps)

            # final: DMA accumulated output tiles
            for st in range(NKT):
                for oc in range(2):
                    nc.sync.dma_start(
                        out=OP[st * 128:(st + 1) * 128,
                               oc * 512:(oc + 1) * 512],
                        in_=out_acc[st][oc])


_NC_CACHE = None
_last_in_maps = None


def _get_nc():
    global _NC_CACHE
    if _NC_CACHE is None:
        _NC_CACHE = build()
    return _NC_CACHE


def kernel(Q, K, V, W_Q, b_Q, W_K, b_K, W_V, b_V, W_O, b_O):
    global _last_in_maps
    Q = np.asarray(Q, dtype=np.float32)
    K = np.asarray(K, dtype=np.float32)
    V = np.asarray(V, dtype=np.float32)
    nc = _get_nc()

    XqTs = [prep(Q[b].T, PROJ_DT) for b in range(B)]
    XkTs = [prep(K[b].T, PROJ_DT) for b in range(B)]
    XvTs = [prep(V[b].T, PROJ_DT) for b in range(B)]
    Wqs = [prep(np.asarray(W_Q)[:, hg * C:(hg + 1) * C], PROJ_DT)
           for hg in range(2)]
    Wks = [prep(np.asarray(W_K)[:, hg * C:(hg + 1) * C], PROJ_DT)
           for hg in range(2)]
    Wvs = [prep(np.asarray(W_V)[:, hg * C:(hg + 1) * C], PROJ_DT)
           for hg in range(2)]
    Wos = [prep(np.asarray(W_O)[hg * C:(hg + 1) * C, :], OUT_DT)
           for hg in range(2)]
    bqs = [np.ascontiguousarray(np.asarray(b_Q, dtype=np.float32)[hg * C:(hg + 1) * C])
           for hg in range(2)]
    bks = [np.ascontiguousarray(np.asarray(b_K, dtype=np.float32)[hg * C:(hg + 1) * C])
           for hg in range(2)]
    bvs = [np.ascontiguousarray(np.asarray(b_V, dtype=np.float32)[hg * C:(hg + 1) * C])
           for hg in range(2)]

    in_maps = []
    for c in range(N_CORES):
        b, hg = c // 2, c % 2
        in_maps.append({
            "XqT": XqTs[b], "XkT": XkTs[b], "XvT": XvTs[b],
            "Wq": Wqs[hg], "Wk": Wks[hg], "Wv": Wvs[hg], "Wo": Wos[hg],
            "bq": bqs[hg], "bk": bks[hg], "bv": bvs[hg],
        })
    _last_in_maps = in_maps
    res = run_bass_kernel_spmd(nc, in_maps, list(range(N_CORES)))
    out = np.empty((B, S, D), dtype=np.float32)
    bO = np.asarray(b_O, dtype=np.float32)
    for b in range(B):
        out[b] = res.results[2 * b]["OP"] + res.results[2 * b + 1]["OP"] + bO
    return out
